# revision 50
# baseline (speedup 1.0000x reference)
"""Multi-head attention (RoPE, causal) Trainium2 kernel, SPMD over 8 NeuronCores.

Problem: x[2,2048,1024] @ {W_q,W_k,W_v}[1024,1024] -> 16-head causal attention
with RoPE -> @ W_o[1024,1024].

Sharding (batch x heads): core c handles batch b=c//4 and head group g=c%4
(4 heads = 256 of the 1024 qkv dims). Each core computes its heads' QKV
projections, RoPE, causal attention, and a partial out-projection
(ctx_g @ W_o[256g:256g+256, :]). The host sums the 4 partials per batch
(unshard of a partial-sum sharding) and transposes back.

On-device layout is fully transposed ([feature, seq]) so no transposes are
needed anywhere: scores are computed as scoresT[k,q] = K^T.T @ Q^T, the
softmax denominator falls out of the AV matmul via a ones-column appended to
V, and the out-projection consumes ctxT directly.

Two structural ideas on top of the plain pipeline:

1. Causal narrowing. For a diagonal key-block j of a query block, only
   q >= 128j is reachable, so the score matmuls, the exp, and the AV
   matmuls all restrict to that column range, and the mask multiply
   shrinks to the single [128, 128] triangular band (the same lower
   triangle for every j).

2. PE-filler interleave. The attention inner loop is scalar-engine bound
   (exp over [128, 2x512] costs ~1.1us vs ~0.6us of PE work per key
   block), so independent PE work - next block's QKV projections and the
   previous block's out-projection - is emitted *between* key-block
   steps. That keeps the PE dense (no HAM re-throttle) and hides the
   projection cost entirely inside the attention phases.
"""

import numpy as np
import ml_dtypes

B = 2
S = 2048
D = 1024
H = 16
HD = 64
N_CORES = 8
H_PER_CORE = 4
DQ = H_PER_CORE * HD  # 256 qkv dims per core
N_DC = D // 128  # 8 contraction chunks
N_SB = S // 512  # 4 seq blocks of 512
N_KB = S // 128  # 16 key blocks of 128
THETA = 10000.0

_CACHED = None


def _build_kernel():
    import concourse.bass as bass
    import concourse.mybir as mybir
    import concourse.tile as tile
    from concourse import bacc

    f32 = mybir.dt.float32
    bf16 = mybir.dt.bfloat16

    nc = bacc.Bacc(None, target_bir_lowering=False, num_devices=N_CORES)

    # all inputs are pre-arranged on the host into the exact SBUF layout so
    # every load is one fully contiguous DMA (strided loads run ~10x slower)
    xT = nc.dram_tensor("xT", [D, S], bf16, kind="ExternalInput")
    wq = nc.dram_tensor("wq", [128, N_DC, DQ], bf16, kind="ExternalInput")
    wk = nc.dram_tensor("wk", [128, N_DC, DQ], bf16, kind="ExternalInput")
    wv = nc.dram_tensor("wv", [128, N_DC, DQ], bf16, kind="ExternalInput")
    wo = nc.dram_tensor("wo", [128, 2, D], bf16, kind="ExternalInput")
    # chunk-major so each 512-column chunk is one contiguous DMA
    cosT = nc.dram_tensor("cosT", [N_SB, 128, 512], f32, kind="ExternalInput")
    sinT = nc.dram_tensor("sinT", [N_SB, 128, 512], f32, kind="ExternalInput")
    # masks[k, 128*h + c] = 1.0 if k <= c else 0 (h=0,1 same): the triangular
    # band for a diagonal 128-key block, duplicated for the two heads of a
    # chunk (all diagonal blocks share the same band after narrowing)
    masks = nc.dram_tensor("masks", [128, 256], bf16, kind="ExternalInput")
    yT = nc.dram_tensor("yT", [D, S], bf16, kind="ExternalOutput")

    with tile.TileContext(nc) as tc:
        with (
            tc.tile_pool(name="persist", bufs=1) as persist,
            tc.tile_pool(name="attn", bufs=8) as attn_pool,
            tc.tile_pool(name="rope", bufs=4) as rope_pool,
            tc.tile_pool(name="small", bufs=4) as small_pool,
            tc.tile_pool(name="yout", bufs=3) as yout_pool,
            tc.tile_pool(name="dram", bufs=1, space="DRAM") as dram_pool,
            tc.tile_pool(name="psA", bufs=2, space="PSUM") as psA,  # scores 2-bank
            tc.tile_pool(name="psB", bufs=2, space="PSUM") as psB,  # ctx accum
            tc.tile_pool(name="psC", bufs=2, space="PSUM") as psC,  # proj/y
        ):
            # ---------------- input DMA ----------------
            # few, large DMAs on the sync+gpsimd queues; ordered so the
            # tensors the pipeline needs first (wq, xt, cos/sin first half,
            # mask) land first instead of queueing behind the rest
            # ordered by first use: the critical sequence to reach steady
            # state is wq+xt+wk (projections), cos/sin chunk 0 + mask
            # (rope + first exp); wv/wo and the later cos/sin chunks ride
            # on the scalar queue / sync tail
            wq_sb = persist.tile([128, N_DC, DQ], bf16, tag="wq")
            nc.sync.dma_start(out=wq_sb[:], in_=wq[:])
            xt_sb = [
                persist.tile([128, S], bf16, tag=f"xt{dc}", name=f"xt{dc}")
                for dc in range(N_DC)
            ]
            cos_sb = persist.tile([128, N_SB, 512], f32, tag="cos")
            sin_sb = persist.tile([128, N_SB, 512], f32, tag="sin")
            mask_sb = persist.tile([128, 2, 128], bf16, tag="mask")
            wk_sb = persist.tile([128, N_DC, DQ], bf16, tag="wk")
            wv_sb = persist.tile([128, N_DC, DQ], bf16, tag="wv")
            wo_sb = persist.tile([128, 2, D], bf16, tag="wo")
            for dc in (0, 1, 2, 3):
                nc.sync.dma_start(
                    out=xt_sb[dc][:], in_=xT[128 * dc : 128 * (dc + 1), :]
                )
            nc.sync.dma_start(out=cos_sb[:, 0, :], in_=cosT[0])
            nc.sync.dma_start(out=sin_sb[:, 0, :], in_=sinT[0])
            nc.sync.dma_start(
                out=mask_sb[:], in_=masks.rearrange("p (h c) -> p h c", h=2)
            )
            nc.sync.dma_start(out=cos_sb[:, 1, :], in_=cosT[1])
            nc.sync.dma_start(out=sin_sb[:, 1, :], in_=sinT[1])
            nc.scalar.dma_start(out=wk_sb[:], in_=wk[:])
            for dc in (4, 5, 6, 7):
                nc.scalar.dma_start(
                    out=xt_sb[dc][:], in_=xT[128 * dc : 128 * (dc + 1), :]
                )
            nc.scalar.dma_start(out=wv_sb[:], in_=wv[:])
            nc.gpsimd.dma_start(out=wo_sb[:], in_=wo[:])
            for c in range(2, N_SB):
                nc.gpsimd.dma_start(out=cos_sb[:, c, :], in_=cosT[c])
                nc.gpsimd.dma_start(out=sin_sb[:, c, :], in_=sinT[c])

            # PE warm-up: the HAM clock gate needs ~3.4us of sustained
            # activity to lift the PE to 2.4GHz; run throwaway matmuls on a
            # memset constant tile so they start before any input lands
            cst_sb = persist.tile([128, DQ], bf16, tag="cst")
            nc.vector.memset(cst_sb[:], 0.5)
            warm0 = psA.tile([128, DQ], f32, tag="score", name="warm0")
            for wi in range(24):
                nc.tensor.matmul(
                    warm0[:],
                    cst_sb[:, 0:128],
                    cst_sb[:],
                    start=True,
                    stop=True,
                )

            # persistent intermediates
            qT_sb = persist.tile([128, 2, S], bf16, tag="qT")  # [64h..., cc, s]
            kT_sb = persist.tile([128, 2, S], bf16, tag="kT")
            v_sb = persist.tile([128, N_KB, H_PER_CORE, HD + 1], bf16, tag="v")
            nc.vector.memset(v_sb[:, :, :, HD : HD + 1], 1.0)
            ctxT_sb = persist.tile([128, 2, S], bf16, tag="ctxT")  # unnormalized
            # denominators staged on one partition (engine writes must start at
            # partition 0/32/64/96); chunk qb*4+hh holds head hh, block qb
            stage_sb = persist.tile([1, H_PER_CORE * S], f32, tag="stage")
            recip_dram = dram_pool.tile([N_SB, H_PER_CORE, 512], bf16, tag="rdram")

            # ---------------- helpers ----------------
            def rope(src_ps, dst_sb, cc, sb):
                """dst = src*cos + rotate_half(src)*sin, fp32 in, bf16 out.

                The rotate-half partition shift is done by small SBUF->SBUF
                DMAs (a [32,512] DVE op costs as much as a [128,512] one, so
                quarter-sized DVE ops waste 3/4 of the lanes; DMA engines are
                otherwise idle).
                """
                t1 = rope_pool.tile([128, 512], bf16, tag="ropeA", name="t1")
                nc.vector.tensor_mul(t1[:], src_ps[:], cos_sb[:, sb, :])
                # sin table is pre-shifted on the host (sinx[p] =
                # sin_signed[partner(p)]) so this product is computed at the
                # SOURCE rows and only then moved to the partner rows by DMA
                t2p = rope_pool.tile([128, 512], bf16, tag="ropeQ", name="t2p")
                nc.vector.tensor_mul(t2p[:], src_ps[:], sin_sb[:, sb, :])
                ss = slice(512 * sb, 512 * (sb + 1))
                rot = rope_pool.tile([128, 512], bf16, tag="ropeB", name="rot")
                for quarter in range(4):
                    o = 32 * quarter
                    src_o = o + 32 if quarter % 2 == 0 else o - 32
                    nc.gpsimd.dma_start(
                        out=rot[o : o + 32, :], in_=t2p[src_o : src_o + 32, :]
                    )
                nc.vector.tensor_add(dst_sb[:, cc, ss], t1[:], rot[:])

            def proj_q(cc, sb):
                ss = slice(512 * sb, 512 * (sb + 1))
                q_ps = psC.tile([128, 512], f32, tag="proj", name="q_ps")
                for dc in range(N_DC):
                    nc.tensor.matmul(
                        q_ps[:],
                        wq_sb[:, dc, 128 * cc : 128 * (cc + 1)],
                        xt_sb[dc][:, ss],
                        start=(dc == 0),
                        stop=(dc == N_DC - 1),
                    )
                rope(q_ps, qT_sb, cc, sb)

            def proj_k(cc, sb):
                ss = slice(512 * sb, 512 * (sb + 1))
                k_ps = psC.tile([128, 512], f32, tag="proj", name="k_ps")
                for dc in range(N_DC):
                    nc.tensor.matmul(
                        k_ps[:],
                        wk_sb[:, dc, 128 * cc : 128 * (cc + 1)],
                        xt_sb[dc][:, ss],
                        start=(dc == 0),
                        stop=(dc == N_DC - 1),
                    )
                rope(k_ps, kT_sb, cc, sb)

            def proj_v(sc):
                v_ps = psC.tile([128, DQ], f32, tag="proj", name="v_ps")
                for dc in range(N_DC):
                    nc.tensor.matmul(
                        v_ps[:],
                        xt_sb[dc][:, 128 * sc : 128 * (sc + 1)],
                        wv_sb[:, dc, :],
                        start=(dc == 0),
                        stop=(dc == N_DC - 1),
                    )
                nc.vector.tensor_copy(
                    v_sb[:, sc, :, 0:HD],
                    v_ps[:].rearrange("p (h d) -> p h d", h=H_PER_CORE),
                )

            def out_proj(qb, oc):
                """Partial out-projection for query block qb, output chunk oc."""
                qs = slice(512 * qb, 512 * (qb + 1))
                y_ps = psC.tile([128, 512], f32, tag="proj", name="y_ps")
                for cc in range(2):
                    nc.tensor.matmul(
                        y_ps[:],
                        wo_sb[:, cc, 128 * oc : 128 * (oc + 1)],
                        ctxT_sb[:, cc, qs],
                        start=(cc == 0),
                        stop=(cc == 1),
                    )
                y_sb = yout_pool.tile([128, 512], bf16, tag="y", name="y_sb")
                nc.vector.tensor_copy(y_sb[:], y_ps[:])
                nc.sync.dma_start(
                    out=yT[128 * oc : 128 * (oc + 1), qs], in_=y_sb[:]
                )

            def attention(cc, qb, fillers=()):
                """Causal attention for head pair cc, query block qb.

                Per k-block: two score matmuls (head h in PE row-group h) into
                one [128,1024] PSUM tile, one exp over both heads, a
                triangular band mask on diagonal blocks, then (one k-block
                delayed) the two AV matmuls accumulating ctx+denominator via
                the ones column. Diagonal blocks narrow everything to the
                causally reachable q-range.

                `fillers` is a list of callables emitting independent PE
                work, spread between k-blocks to cover the exp-bound inner
                loop (the PE would otherwise idle ~0.5us per k-block).
                """
                qs0 = 512 * qb
                nkb = 4 * qb + 4
                fillers = list(fillers)
                emit_at = {}
                for i, f in enumerate(fillers):
                    kb_i = min(nkb - 1, 1 + (i * nkb) // max(1, len(fillers)))
                    emit_at.setdefault(kb_i, []).append(f)
                ctx_ps = [
                    psB.tile([HD + 1, 512], f32, tag="ctx", name=f"ctx{h}")
                    for h in range(2)
                ]
                pending = None  # (kb, lo, a_t) whose AV matmuls haven't run
                for kb in range(nkb):
                    diag = kb >= 4 * qb
                    lo = 128 * (kb - 4 * qb) if diag else 0
                    s_ps = psA.tile([128, 1024], f32, tag="score", name="s_ps")
                    for h in range(2):
                        hp = slice(64 * h, 64 * (h + 1))
                        nc.tensor.matmul(
                            s_ps[:, 512 * h + lo : 512 * (h + 1)],
                            kT_sb[hp, cc, 128 * kb : 128 * (kb + 1)],
                            qT_sb[hp, cc, qs0 + lo : qs0 + 512],
                            start=True,
                            stop=True,
                        )
                    a_t = attn_pool.tile(
                        [128, 2, 512], bf16, tag="attnT", name="a_t"
                    )
                    nc.scalar.activation(
                        a_t[:, :, lo:512],
                        s_ps[:].rearrange("p (h q) -> p h q", h=2)[:, :, lo:512],
                        mybir.ActivationFunctionType.Exp,
                        scale=float(1.0 / np.sqrt(HD)),
                    )
                    if diag:
                        nc.vector.tensor_mul(
                            a_t[:, :, lo : lo + 128],
                            a_t[:, :, lo : lo + 128],
                            mask_sb[:],
                        )
                    if pending is not None:
                        pkb, plo, p_t = pending
                        for h in range(2):
                            nc.tensor.matmul(
                                ctx_ps[h][:, plo:512],
                                v_sb[:, pkb, 2 * cc + h, :],
                                p_t[:, h, plo:512],
                                start=(pkb == 0),
                                stop=False,
                            )
                    for f in emit_at.get(kb, ()):
                        f()

                    pending = (kb, lo, a_t)
                pkb, plo, p_t = pending
                for h in range(2):
                    nc.tensor.matmul(
                        ctx_ps[h][:, plo:512],
                        v_sb[:, pkb, 2 * cc + h, :],
                        p_t[:, h, plo:512],
                        start=(pkb == 0),
                        stop=True,
                    )
                # stage denominators first (the normalization chain hangs
                # off them), then the bulk ctx copies
                r0 = qb * H_PER_CORE + 2 * cc
                nc.vector.tensor_copy(
                    stage_sb[0:1, 512 * r0 : 512 * (r0 + 1)],
                    ctx_ps[0][HD : HD + 1, :],
                )
                nc.scalar.copy(
                    stage_sb[0:1, 512 * (r0 + 1) : 512 * (r0 + 2)],
                    ctx_ps[1][HD : HD + 1, :],
                )
                for h in range(2):
                    nc.vector.tensor_copy(
                        ctxT_sb[64 * h : 64 * (h + 1), cc, qs0 : qs0 + 512],
                        ctx_ps[h][0:HD, :],
                    )

            def normalize(cc, qb, tail=False):
                """Reciprocal + broadcast + scale for head pair cc, block qb."""
                # repartition [1, 1024] -> [8, 128] so reciprocal is cheap
                # (reciprocal cost scales with free size only) and so the DVE
                # queue does not block on the scalar engine's stage copy (the
                # wait happens on the sync queue instead)
                base = (qb * H_PER_CORE + 2 * cc) * 512
                den_q = small_pool.tile([8, 128], f32, tag="den_q", name="den_q")
                # the final block's chain rides the scalar queue (empty once
                # the last exp retires) instead of the busy sync queue
                deng = nc.scalar if tail else nc.sync
                deng.dma_start(
                    out=den_q[:], in_=stage_sb[0:1, base : base + 1024]
                )
                rec_q = small_pool.tile([8, 128], bf16, tag="rec_q", name="rec_q")
                with nc.allow_low_precision(
                    reason="bf16 softmax denom matches bf16 attn weights"
                ):
                    nc.vector.reciprocal(rec_q[:], den_q[:])
                deng.dma_start(
                    out=recip_dram[qb, 2 * cc : 2 * cc + 2, :], in_=rec_q[:]
                )
                qs = slice(512 * qb, 512 * (qb + 1))
                bc_sb = small_pool.tile([128, 512], bf16, tag="bcast", name="bc_sb")
                for h in range(2):
                    row = recip_dram[qb, 2 * cc + h, :]
                    bcast = bass.AP(
                        tensor=row.tensor,
                        offset=row.offset,
                        ap=[[0, 64]] + list(row.ap)[-1:],
                    )
                    deng.dma_start(
                        out=bc_sb[64 * h : 64 * (h + 1), :], in_=bcast
                    )
                nc.vector.tensor_mul(
                    ctxT_sb[:, cc, qs], ctxT_sb[:, cc, qs], bc_sb[:]
                )

            # ---------------- main pipeline ----------------
            # prologue: just what attention(0, 0) needs; the rest of block
            # 0's projections ride as fillers inside attention(0, 0)
            proj_q(0, 0)
            proj_k(0, 0)
            for sc in range(4):
                proj_v(sc)

            for sb in range(N_SB):
                nxt = sb + 1
                # fillers for attention(0, sb): next block's cc=0 q/k
                # projections + v, then the previous block's out-projection
                fill0 = []
                if sb == 0:
                    fill0 += [lambda: proj_q(1, 0), lambda: proj_k(1, 0)]
                if nxt < N_SB:
                    fill0 += [
                        lambda s=nxt: proj_q(0, s),
                        lambda s=nxt: proj_k(0, s),
                        lambda s=nxt: proj_v(4 * s + 0),
                        lambda s=nxt: proj_v(4 * s + 1),
                    ]
                if sb >= 1:
                    fill0 += [
                        (lambda q=sb - 1, oc=oc: out_proj(q, oc))
                        for oc in range(0, 4)
                    ]
                attention(0, sb, fillers=fill0)
                normalize(0, sb)

                fill1 = []
                if nxt < N_SB:
                    fill1 += [
                        lambda s=nxt: proj_q(1, s),
                        lambda s=nxt: proj_k(1, s),
                        lambda s=nxt: proj_v(4 * s + 2),
                        lambda s=nxt: proj_v(4 * s + 3),
                    ]
                if sb >= 1:
                    fill1 += [
                        (lambda q=sb - 1, oc=oc: out_proj(q, oc))
                        for oc in range(4, N_DC)
                    ]
                attention(1, sb, fillers=fill1)
                normalize(1, sb, tail=(sb == N_SB - 1))

            for oc in range(N_DC):
                out_proj(N_SB - 1, oc)

    nc.compile()
    return nc


def _rope_tables():
    inv_freq = (
        1.0 / (THETA ** (np.arange(0, HD, 2, dtype=np.float32) / HD))
    ).astype(np.float32)
    pos = np.arange(S, dtype=np.float32)
    ang = pos[:, None] * inv_freq[None, :]  # [S, 32]
    cos_half = np.cos(ang).astype(np.float32).T  # [32, S]
    sin_half = np.sin(ang).astype(np.float32).T
    # per-head 64 rows: cos rows duplicated. The sin table is PRE-SHIFTED:
    # row p holds sin_signed[partner(p)] (partner = rotate-half swap), so the
    # kernel multiplies at the source rows and a plain partition-shift DMA
    # finishes rotate-half: sinx per head = (+sin | -sin).
    cos64 = np.concatenate([cos_half, cos_half], axis=0)
    sinx64 = np.concatenate([sin_half, -sin_half], axis=0)
    cosT = np.concatenate([cos64, cos64], axis=0)  # [128, S] two heads
    sinT = np.concatenate([sinx64, sinx64], axis=0)

    def chunked(t):  # [128, S] -> [N_SB, 128, 512] chunk-major
        return np.ascontiguousarray(
            t.reshape(128, S // 512, 512).transpose(1, 0, 2)
        )

    return chunked(cosT), chunked(sinT)


def _masks():
    k = np.arange(128)[:, None]
    c = np.arange(128)[None, :]
    tri = (k <= c).astype(ml_dtypes.bfloat16)  # [128, 128] band triangle
    return np.ascontiguousarray(np.concatenate([tri, tri], axis=1))


def kernel(x, W_q, W_k, W_v, W_o):
    global _CACHED
    from concourse.bass_utils import run_bass_kernel_spmd

    if _CACHED is None:
        _CACHED = _build_kernel()
    nc = _CACHED

    bf = ml_dtypes.bfloat16
    cosT, sinT = _rope_tables()
    masks = _masks()
    x = np.asarray(x)
    W_q, W_k, W_v, W_o = (np.asarray(w) for w in (W_q, W_k, W_v, W_o))
    xT = [np.ascontiguousarray(x[b].T).astype(bf) for b in range(B)]

    def chunk_major(w):  # [D_in, n] -> [128, D_in//128, n] (SBUF layout)
        w = np.asarray(w, dtype=bf)
        return np.ascontiguousarray(
            w.reshape(w.shape[0] // 128, 128, w.shape[1]).transpose(1, 0, 2)
        )

    in_maps = []
    for c in range(N_CORES):
        b, g = divmod(c, 4)
        cols = slice(DQ * g, DQ * (g + 1))
        in_maps.append(
            {
                "xT": xT[b],
                "wq": chunk_major(W_q[:, cols]),
                "wk": chunk_major(W_k[:, cols]),
                "wv": chunk_major(W_v[:, cols]),
                "wo": chunk_major(W_o[cols, :]),
                "cosT": cosT,
                "sinT": sinT,
                "masks": masks,
            }
        )

    res = run_bass_kernel_spmd(nc, in_maps, core_ids=list(range(N_CORES)))
    kernel.last_results = res

    y = np.empty((B, S, D), dtype=np.float32)
    for b in range(B):
        acc = res.results[4 * b]["yT"].astype(np.float32)
        for g in range(1, 4):
            acc += res.results[4 * b + g]["yT"].astype(np.float32)
        y[b] = acc.T
    return y


# revision 51
# speedup vs baseline: 1.0274x; 1.0274x over previous
"""Multi-head attention (RoPE, causal) Trainium2 kernel, SPMD over 8 NeuronCores.

Problem: x[2,2048,1024] @ {W_q,W_k,W_v}[1024,1024] -> 16-head causal attention
with RoPE -> @ W_o[1024,1024].

Sharding (batch x heads): core c handles batch b=c//4 and head group g=c%4
(4 heads = 256 of the 1024 qkv dims). Each core computes its heads' QKV
projections, RoPE, causal attention, and a partial out-projection
(ctx_g @ W_o[256g:256g+256, :]). The host sums the 4 partials per batch
(unshard of a partial-sum sharding) and transposes back.

On-device layout is fully transposed ([feature, seq]) so no transposes are
needed anywhere: scores are computed as scoresT[k,q] = K^T.T @ Q^T, the
softmax denominator falls out of the AV matmul via a ones-column appended to
V, and the out-projection consumes ctxT directly.

Two structural ideas on top of the plain pipeline:

1. Causal narrowing. For a diagonal key-block j of a query block, only
   q >= 128j is reachable, so the score matmuls, the exp, and the AV
   matmuls all restrict to that column range, and the mask multiply
   shrinks to the single [128, 128] triangular band (the same lower
   triangle for every j).

2. PE-filler interleave. The attention inner loop is scalar-engine bound
   (exp over [128, 2x512] costs ~1.1us vs ~0.6us of PE work per key
   block), so independent PE work - next block's QKV projections and the
   previous block's out-projection - is emitted *between* key-block
   steps. That keeps the PE dense (no HAM re-throttle) and hides the
   projection cost entirely inside the attention phases.
"""

import numpy as np
import ml_dtypes

B = 2
S = 2048
D = 1024
H = 16
HD = 64
N_CORES = 8
H_PER_CORE = 4
DQ = H_PER_CORE * HD  # 256 qkv dims per core
N_DC = D // 128  # 8 contraction chunks
N_SB = S // 512  # 4 seq blocks of 512
N_KB = S // 128  # 16 key blocks of 128
THETA = 10000.0

_CACHED = None


def _build_kernel():
    import concourse.bass as bass
    import concourse.mybir as mybir
    import concourse.tile as tile
    from concourse import bacc

    f32 = mybir.dt.float32
    bf16 = mybir.dt.bfloat16

    nc = bacc.Bacc(None, target_bir_lowering=False, num_devices=N_CORES)

    # all inputs are pre-arranged on the host into the exact SBUF layout so
    # every load is one fully contiguous DMA (strided loads run ~10x slower)
    xT = nc.dram_tensor("xT", [D, S], bf16, kind="ExternalInput")
    wq = nc.dram_tensor("wq", [128, N_DC, DQ], bf16, kind="ExternalInput")
    wk = nc.dram_tensor("wk", [128, N_DC, DQ], bf16, kind="ExternalInput")
    wv = nc.dram_tensor("wv", [128, N_DC, DQ], bf16, kind="ExternalInput")
    wo = nc.dram_tensor("wo", [128, 2, D], bf16, kind="ExternalInput")
    # chunk-major so each 512-column chunk is one contiguous DMA
    cosT = nc.dram_tensor("cosT", [N_SB, 128, 512], f32, kind="ExternalInput")
    sinT = nc.dram_tensor("sinT", [N_SB, 128, 512], f32, kind="ExternalInput")
    # masks[k, 128*h + c] = 1.0 if k <= c else 0 (h=0,1 same): the triangular
    # band for a diagonal 128-key block, duplicated for the two heads of a
    # chunk (all diagonal blocks share the same band after narrowing)
    masks = nc.dram_tensor("masks", [128, 256], bf16, kind="ExternalInput")
    yT = nc.dram_tensor("yT", [D, S], bf16, kind="ExternalOutput")

    with tile.TileContext(nc) as tc:
        with (
            tc.tile_pool(name="persist", bufs=1) as persist,
            tc.tile_pool(name="attn", bufs=8) as attn_pool,
            tc.tile_pool(name="rope", bufs=4) as rope_pool,
            tc.tile_pool(name="small", bufs=4) as small_pool,
            tc.tile_pool(name="yout", bufs=3) as yout_pool,
            tc.tile_pool(name="dram", bufs=1, space="DRAM") as dram_pool,
            tc.tile_pool(name="psA", bufs=2, space="PSUM") as psA,  # scores 2-bank
            tc.tile_pool(name="psB", bufs=2, space="PSUM") as psB,  # ctx accum
            tc.tile_pool(name="psC", bufs=2, space="PSUM") as psC,  # proj/y
        ):
            # ---------------- input DMA ----------------
            # few, large DMAs on the sync+gpsimd queues; ordered so the
            # tensors the pipeline needs first (wq, xt, cos/sin first half,
            # mask) land first instead of queueing behind the rest
            # ordered by first use: the critical sequence to reach steady
            # state is wq+xt+wk (projections), cos/sin chunk 0 + mask
            # (rope + first exp); wv/wo and the later cos/sin chunks ride
            # on the scalar queue / sync tail
            wq_sb = persist.tile([128, N_DC, DQ], bf16, tag="wq")
            nc.sync.dma_start(out=wq_sb[:], in_=wq[:])
            xt_sb = [
                persist.tile([128, S], bf16, tag=f"xt{dc}", name=f"xt{dc}")
                for dc in range(N_DC)
            ]
            cos_sb = persist.tile([128, N_SB, 512], f32, tag="cos")
            sin_sb = persist.tile([128, N_SB, 512], f32, tag="sin")
            mask_sb = persist.tile([128, 2, 128], bf16, tag="mask")
            wk_sb = persist.tile([128, N_DC, DQ], bf16, tag="wk")
            wv_sb = persist.tile([128, N_DC, DQ], bf16, tag="wv")
            wo_sb = persist.tile([128, 2, D], bf16, tag="wo")
            nc.gpsimd.dma_start(out=cos_sb[:, 0, :], in_=cosT[0])
            nc.gpsimd.dma_start(out=sin_sb[:, 0, :], in_=sinT[0])
            nc.gpsimd.dma_start(
                out=mask_sb[:], in_=masks.rearrange("p (h c) -> p h c", h=2)
            )
            for dc in (1, 3, 5, 7):
                nc.gpsimd.dma_start(
                    out=xt_sb[dc][:], in_=xT[128 * dc : 128 * (dc + 1), :]
                )
            for dc in (0, 2):
                nc.sync.dma_start(
                    out=xt_sb[dc][:], in_=xT[128 * dc : 128 * (dc + 1), :]
                )
            nc.sync.dma_start(out=wk_sb[:], in_=wk[:])
            for dc in (4, 6):
                nc.sync.dma_start(
                    out=xt_sb[dc][:], in_=xT[128 * dc : 128 * (dc + 1), :]
                )
            for c in range(1, N_SB):
                nc.sync.dma_start(out=cos_sb[:, c, :], in_=cosT[c])
                nc.sync.dma_start(out=sin_sb[:, c, :], in_=sinT[c])
            nc.scalar.dma_start(out=wv_sb[:], in_=wv[:])
            nc.scalar.dma_start(out=wo_sb[:], in_=wo[:])

            # PE warm-up: the HAM clock gate needs ~3.4us of sustained
            # activity to lift the PE to 2.4GHz; run throwaway matmuls on a
            # memset constant tile so they start before any input lands
            cst_sb = persist.tile([128, DQ], bf16, tag="cst")
            nc.vector.memset(cst_sb[:], 0.5)
            warm0 = psA.tile([128, DQ], f32, tag="score", name="warm0")
            for wi in range(24):
                nc.tensor.matmul(
                    warm0[:],
                    cst_sb[:, 0:128],
                    cst_sb[:],
                    start=True,
                    stop=True,
                )

            # persistent intermediates
            qT_sb = persist.tile([128, 2, S], bf16, tag="qT")  # [64h..., cc, s]
            kT_sb = persist.tile([128, 2, S], bf16, tag="kT")
            v_sb = persist.tile([128, N_KB, H_PER_CORE, HD + 1], bf16, tag="v")
            nc.vector.memset(v_sb[:, :, :, HD : HD + 1], 1.0)
            ctxT_sb = persist.tile([128, 2, S], bf16, tag="ctxT")  # unnormalized
            # denominators staged on one partition (engine writes must start at
            # partition 0/32/64/96); chunk qb*4+hh holds head hh, block qb
            stage_sb = persist.tile([1, H_PER_CORE * S], f32, tag="stage")
            recip_dram = dram_pool.tile([N_SB, H_PER_CORE, 512], bf16, tag="rdram")

            # ---------------- helpers ----------------
            def rope(src_ps, dst_sb, cc, sb):
                """dst = src*cos + rotate_half(src)*sin, fp32 in, bf16 out.

                The rotate-half partition shift is done by small SBUF->SBUF
                DMAs (a [32,512] DVE op costs as much as a [128,512] one, so
                quarter-sized DVE ops waste 3/4 of the lanes; DMA engines are
                otherwise idle).
                """
                t1 = rope_pool.tile([128, 512], bf16, tag="ropeA", name="t1")
                nc.vector.tensor_mul(t1[:], src_ps[:], cos_sb[:, sb, :])
                # sin table is pre-shifted on the host (sinx[p] =
                # sin_signed[partner(p)]) so this product is computed at the
                # SOURCE rows and only then moved to the partner rows by DMA
                t2p = rope_pool.tile([128, 512], bf16, tag="ropeQ", name="t2p")
                nc.vector.tensor_mul(t2p[:], src_ps[:], sin_sb[:, sb, :])
                ss = slice(512 * sb, 512 * (sb + 1))
                rot = rope_pool.tile([128, 512], bf16, tag="ropeB", name="rot")
                for quarter in range(4):
                    o = 32 * quarter
                    src_o = o + 32 if quarter % 2 == 0 else o - 32
                    nc.gpsimd.dma_start(
                        out=rot[o : o + 32, :], in_=t2p[src_o : src_o + 32, :]
                    )
                nc.vector.tensor_add(dst_sb[:, cc, ss], t1[:], rot[:])

            def proj_q(cc, sb):
                ss = slice(512 * sb, 512 * (sb + 1))
                q_ps = psC.tile([128, 512], f32, tag="proj", name="q_ps")
                for dc in range(N_DC):
                    nc.tensor.matmul(
                        q_ps[:],
                        wq_sb[:, dc, 128 * cc : 128 * (cc + 1)],
                        xt_sb[dc][:, ss],
                        start=(dc == 0),
                        stop=(dc == N_DC - 1),
                    )
                rope(q_ps, qT_sb, cc, sb)

            def proj_k(cc, sb):
                ss = slice(512 * sb, 512 * (sb + 1))
                k_ps = psC.tile([128, 512], f32, tag="proj", name="k_ps")
                for dc in range(N_DC):
                    nc.tensor.matmul(
                        k_ps[:],
                        wk_sb[:, dc, 128 * cc : 128 * (cc + 1)],
                        xt_sb[dc][:, ss],
                        start=(dc == 0),
                        stop=(dc == N_DC - 1),
                    )
                rope(k_ps, kT_sb, cc, sb)

            def proj_v(sc):
                v_ps = psC.tile([128, DQ], f32, tag="proj", name="v_ps")
                for dc in range(N_DC):
                    nc.tensor.matmul(
                        v_ps[:],
                        xt_sb[dc][:, 128 * sc : 128 * (sc + 1)],
                        wv_sb[:, dc, :],
                        start=(dc == 0),
                        stop=(dc == N_DC - 1),
                    )
                nc.vector.tensor_copy(
                    v_sb[:, sc, :, 0:HD],
                    v_ps[:].rearrange("p (h d) -> p h d", h=H_PER_CORE),
                )

            def out_proj(qb, oc):
                """Partial out-projection for query block qb, output chunk oc."""
                qs = slice(512 * qb, 512 * (qb + 1))
                y_ps = psC.tile([128, 512], f32, tag="proj", name="y_ps")
                for cc in range(2):
                    nc.tensor.matmul(
                        y_ps[:],
                        wo_sb[:, cc, 128 * oc : 128 * (oc + 1)],
                        ctxT_sb[:, cc, qs],
                        start=(cc == 0),
                        stop=(cc == 1),
                    )
                y_sb = yout_pool.tile([128, 512], bf16, tag="y", name="y_sb")
                nc.vector.tensor_copy(y_sb[:], y_ps[:])
                nc.sync.dma_start(
                    out=yT[128 * oc : 128 * (oc + 1), qs], in_=y_sb[:]
                )

            def attention(cc, qb, fillers=()):
                """Causal attention for head pair cc, query block qb.

                Per k-block: two score matmuls (head h in PE row-group h) into
                one [128,1024] PSUM tile, one exp over both heads, a
                triangular band mask on diagonal blocks, then (one k-block
                delayed) the two AV matmuls accumulating ctx+denominator via
                the ones column. Diagonal blocks narrow everything to the
                causally reachable q-range.

                `fillers` is a list of callables emitting independent PE
                work, spread between k-blocks to cover the exp-bound inner
                loop (the PE would otherwise idle ~0.5us per k-block).
                """
                qs0 = 512 * qb
                nkb = 4 * qb + 4
                fillers = list(fillers)
                emit_at = {}
                for i, f in enumerate(fillers):
                    kb_i = min(nkb - 1, 1 + (i * nkb) // max(1, len(fillers)))
                    emit_at.setdefault(kb_i, []).append(f)
                ctx_ps = [
                    psB.tile([HD + 1, 512], f32, tag="ctx", name=f"ctx{h}")
                    for h in range(2)
                ]
                pending = None  # (kb, lo, a_t) whose AV matmuls haven't run
                for kb in range(nkb):
                    diag = kb >= 4 * qb
                    lo = 128 * (kb - 4 * qb) if diag else 0
                    s_ps = psA.tile([128, 1024], f32, tag="score", name="s_ps")
                    for h in range(2):
                        hp = slice(64 * h, 64 * (h + 1))
                        nc.tensor.matmul(
                            s_ps[:, 512 * h + lo : 512 * (h + 1)],
                            kT_sb[hp, cc, 128 * kb : 128 * (kb + 1)],
                            qT_sb[hp, cc, qs0 + lo : qs0 + 512],
                            start=True,
                            stop=True,
                        )
                    a_t = attn_pool.tile(
                        [128, 2, 512], bf16, tag="attnT", name="a_t"
                    )
                    nc.scalar.activation(
                        a_t[:, :, lo:512],
                        s_ps[:].rearrange("p (h q) -> p h q", h=2)[:, :, lo:512],
                        mybir.ActivationFunctionType.Exp,
                        scale=float(1.0 / np.sqrt(HD)),
                    )
                    if diag:
                        nc.vector.tensor_mul(
                            a_t[:, :, lo : lo + 128],
                            a_t[:, :, lo : lo + 128],
                            mask_sb[:],
                        )
                    if pending is not None:
                        pkb, plo, p_t = pending
                        for h in range(2):
                            nc.tensor.matmul(
                                ctx_ps[h][:, plo:512],
                                v_sb[:, pkb, 2 * cc + h, :],
                                p_t[:, h, plo:512],
                                start=(pkb == 0),
                                stop=False,
                            )
                    for f in emit_at.get(kb, ()):
                        f()

                    pending = (kb, lo, a_t)
                pkb, plo, p_t = pending
                for h in range(2):
                    nc.tensor.matmul(
                        ctx_ps[h][:, plo:512],
                        v_sb[:, pkb, 2 * cc + h, :],
                        p_t[:, h, plo:512],
                        start=(pkb == 0),
                        stop=True,
                    )
                # stage denominators first (the normalization chain hangs
                # off them), then the bulk ctx copies
                r0 = qb * H_PER_CORE + 2 * cc
                nc.vector.tensor_copy(
                    stage_sb[0:1, 512 * r0 : 512 * (r0 + 1)],
                    ctx_ps[0][HD : HD + 1, :],
                )
                nc.scalar.copy(
                    stage_sb[0:1, 512 * (r0 + 1) : 512 * (r0 + 2)],
                    ctx_ps[1][HD : HD + 1, :],
                )
                for h in range(2):
                    nc.vector.tensor_copy(
                        ctxT_sb[64 * h : 64 * (h + 1), cc, qs0 : qs0 + 512],
                        ctx_ps[h][0:HD, :],
                    )

            def normalize(cc, qb, tail=False):
                """Reciprocal + broadcast + scale for head pair cc, block qb."""
                # repartition [1, 1024] -> [8, 128] so reciprocal is cheap
                # (reciprocal cost scales with free size only) and so the DVE
                # queue does not block on the scalar engine's stage copy (the
                # wait happens on the sync queue instead)
                base = (qb * H_PER_CORE + 2 * cc) * 512
                den_q = small_pool.tile([8, 128], f32, tag="den_q", name="den_q")
                # the final block's chain rides the scalar queue (empty once
                # the last exp retires) instead of the busy sync queue
                deng = nc.scalar if tail else nc.sync
                deng.dma_start(
                    out=den_q[:], in_=stage_sb[0:1, base : base + 1024]
                )
                rec_q = small_pool.tile([8, 128], bf16, tag="rec_q", name="rec_q")
                with nc.allow_low_precision(
                    reason="bf16 softmax denom matches bf16 attn weights"
                ):
                    nc.vector.reciprocal(rec_q[:], den_q[:])
                deng.dma_start(
                    out=recip_dram[qb, 2 * cc : 2 * cc + 2, :], in_=rec_q[:]
                )
                qs = slice(512 * qb, 512 * (qb + 1))
                bc_sb = small_pool.tile([128, 512], bf16, tag="bcast", name="bc_sb")
                for h in range(2):
                    row = recip_dram[qb, 2 * cc + h, :]
                    bcast = bass.AP(
                        tensor=row.tensor,
                        offset=row.offset,
                        ap=[[0, 64]] + list(row.ap)[-1:],
                    )
                    deng.dma_start(
                        out=bc_sb[64 * h : 64 * (h + 1), :], in_=bcast
                    )
                nc.vector.tensor_mul(
                    ctxT_sb[:, cc, qs], ctxT_sb[:, cc, qs], bc_sb[:]
                )

            # ---------------- main pipeline ----------------
            # prologue: just what attention(0, 0) needs; the rest of block
            # 0's projections ride as fillers inside attention(0, 0)
            proj_q(0, 0)
            proj_k(0, 0)
            for sc in range(4):
                proj_v(sc)

            for sb in range(N_SB):
                nxt = sb + 1
                # fillers for attention(0, sb): next block's cc=0 q/k
                # projections + v, then the previous block's out-projection
                fill0 = []
                if sb == 0:
                    fill0 += [lambda: proj_q(1, 0), lambda: proj_k(1, 0)]
                if nxt < N_SB:
                    fill0 += [
                        lambda s=nxt: proj_q(0, s),
                        lambda s=nxt: proj_k(0, s),
                        lambda s=nxt: proj_v(4 * s + 0),
                        lambda s=nxt: proj_v(4 * s + 1),
                    ]
                if sb >= 1:
                    fill0 += [
                        (lambda q=sb - 1, oc=oc: out_proj(q, oc))
                        for oc in range(0, 4)
                    ]
                attention(0, sb, fillers=fill0)
                normalize(0, sb)

                fill1 = []
                if nxt < N_SB:
                    fill1 += [
                        lambda s=nxt: proj_q(1, s),
                        lambda s=nxt: proj_k(1, s),
                        lambda s=nxt: proj_v(4 * s + 2),
                        lambda s=nxt: proj_v(4 * s + 3),
                    ]
                if sb >= 1:
                    fill1 += [
                        (lambda q=sb - 1, oc=oc: out_proj(q, oc))
                        for oc in range(4, N_DC)
                    ]
                attention(1, sb, fillers=fill1)
                normalize(1, sb, tail=(sb == N_SB - 1))

            for oc in range(N_DC):
                out_proj(N_SB - 1, oc)

    nc.compile()
    return nc


def _rope_tables():
    inv_freq = (
        1.0 / (THETA ** (np.arange(0, HD, 2, dtype=np.float32) / HD))
    ).astype(np.float32)
    pos = np.arange(S, dtype=np.float32)
    ang = pos[:, None] * inv_freq[None, :]  # [S, 32]
    cos_half = np.cos(ang).astype(np.float32).T  # [32, S]
    sin_half = np.sin(ang).astype(np.float32).T
    # per-head 64 rows: cos rows duplicated. The sin table is PRE-SHIFTED:
    # row p holds sin_signed[partner(p)] (partner = rotate-half swap), so the
    # kernel multiplies at the source rows and a plain partition-shift DMA
    # finishes rotate-half: sinx per head = (+sin | -sin).
    cos64 = np.concatenate([cos_half, cos_half], axis=0)
    sinx64 = np.concatenate([sin_half, -sin_half], axis=0)
    cosT = np.concatenate([cos64, cos64], axis=0)  # [128, S] two heads
    sinT = np.concatenate([sinx64, sinx64], axis=0)

    def chunked(t):  # [128, S] -> [N_SB, 128, 512] chunk-major
        return np.ascontiguousarray(
            t.reshape(128, S // 512, 512).transpose(1, 0, 2)
        )

    return chunked(cosT), chunked(sinT)


def _masks():
    k = np.arange(128)[:, None]
    c = np.arange(128)[None, :]
    tri = (k <= c).astype(ml_dtypes.bfloat16)  # [128, 128] band triangle
    return np.ascontiguousarray(np.concatenate([tri, tri], axis=1))


def kernel(x, W_q, W_k, W_v, W_o):
    global _CACHED
    from concourse.bass_utils import run_bass_kernel_spmd

    if _CACHED is None:
        _CACHED = _build_kernel()
    nc = _CACHED

    bf = ml_dtypes.bfloat16
    cosT, sinT = _rope_tables()
    masks = _masks()
    x = np.asarray(x)
    W_q, W_k, W_v, W_o = (np.asarray(w) for w in (W_q, W_k, W_v, W_o))
    xT = [np.ascontiguousarray(x[b].T).astype(bf) for b in range(B)]

    def chunk_major(w):  # [D_in, n] -> [128, D_in//128, n] (SBUF layout)
        w = np.asarray(w, dtype=bf)
        return np.ascontiguousarray(
            w.reshape(w.shape[0] // 128, 128, w.shape[1]).transpose(1, 0, 2)
        )

    in_maps = []
    for c in range(N_CORES):
        b, g = divmod(c, 4)
        cols = slice(DQ * g, DQ * (g + 1))
        in_maps.append(
            {
                "xT": xT[b],
                "wq": chunk_major(W_q[:, cols]),
                "wk": chunk_major(W_k[:, cols]),
                "wv": chunk_major(W_v[:, cols]),
                "wo": chunk_major(W_o[cols, :]),
                "cosT": cosT,
                "sinT": sinT,
                "masks": masks,
            }
        )

    res = run_bass_kernel_spmd(nc, in_maps, core_ids=list(range(N_CORES)))
    kernel.last_results = res

    y = np.empty((B, S, D), dtype=np.float32)
    for b in range(B):
        acc = res.results[4 * b]["yT"].astype(np.float32)
        for g in range(1, 4):
            acc += res.results[4 * b + g]["yT"].astype(np.float32)
        y[b] = acc.T
    return y


# revision 53
# speedup vs baseline: 1.0345x; 1.0069x over previous
"""Multi-head attention (RoPE, causal) Trainium2 kernel, SPMD over 8 NeuronCores.

Problem: x[2,2048,1024] @ {W_q,W_k,W_v}[1024,1024] -> 16-head causal attention
with RoPE -> @ W_o[1024,1024].

Sharding (batch x heads): core c handles batch b=c//4 and head group g=c%4
(4 heads = 256 of the 1024 qkv dims). Each core computes its heads' QKV
projections, RoPE, causal attention, and a partial out-projection
(ctx_g @ W_o[256g:256g+256, :]). The host sums the 4 partials per batch
(unshard of a partial-sum sharding) and transposes back.

On-device layout is fully transposed ([feature, seq]) so no transposes are
needed anywhere: scores are computed as scoresT[k,q] = K^T.T @ Q^T, the
softmax denominator falls out of the AV matmul via a ones-column appended to
V, and the out-projection consumes ctxT directly.

Two structural ideas on top of the plain pipeline:

1. Causal narrowing. For a diagonal key-block j of a query block, only
   q >= 128j is reachable, so the score matmuls, the exp, and the AV
   matmuls all restrict to that column range, and the mask multiply
   shrinks to the single [128, 128] triangular band (the same lower
   triangle for every j).

2. PE-filler interleave. The attention inner loop is scalar-engine bound
   (exp over [128, 2x512] costs ~1.1us vs ~0.6us of PE work per key
   block), so independent PE work - next block's QKV projections and the
   previous block's out-projection - is emitted *between* key-block
   steps. That keeps the PE dense (no HAM re-throttle) and hides the
   projection cost entirely inside the attention phases.
"""

import numpy as np
import ml_dtypes

B = 2
S = 2048
D = 1024
H = 16
HD = 64
N_CORES = 8
H_PER_CORE = 4
DQ = H_PER_CORE * HD  # 256 qkv dims per core
N_DC = D // 128  # 8 contraction chunks
N_SB = S // 512  # 4 seq blocks of 512
N_KB = S // 128  # 16 key blocks of 128
THETA = 10000.0

_CACHED = None


def _build_kernel():
    import concourse.bass as bass
    import concourse.mybir as mybir
    import concourse.tile as tile
    from concourse import bacc

    f32 = mybir.dt.float32
    bf16 = mybir.dt.bfloat16

    nc = bacc.Bacc(None, target_bir_lowering=False, num_devices=N_CORES)

    # all inputs are pre-arranged on the host into the exact SBUF layout so
    # every load is one fully contiguous DMA (strided loads run ~10x slower)
    xT = nc.dram_tensor("xT", [D, S], bf16, kind="ExternalInput")
    wq = nc.dram_tensor("wq", [128, N_DC, DQ], bf16, kind="ExternalInput")
    wk = nc.dram_tensor("wk", [128, N_DC, DQ], bf16, kind="ExternalInput")
    wv = nc.dram_tensor("wv", [128, N_DC, DQ], bf16, kind="ExternalInput")
    wo = nc.dram_tensor("wo", [128, 2, D], bf16, kind="ExternalInput")
    # chunk-major so each 512-column chunk is one contiguous DMA
    cosT = nc.dram_tensor("cosT", [N_SB, 128, 512], f32, kind="ExternalInput")
    sinT = nc.dram_tensor("sinT", [N_SB, 128, 512], f32, kind="ExternalInput")
    # masks[k, 128*h + c] = 1.0 if k <= c else 0 (h=0,1 same): the triangular
    # band for a diagonal 128-key block, duplicated for the two heads of a
    # chunk (all diagonal blocks share the same band after narrowing)
    masks = nc.dram_tensor("masks", [128, 256], bf16, kind="ExternalInput")
    yT = nc.dram_tensor("yT", [D, S], bf16, kind="ExternalOutput")

    with tile.TileContext(nc) as tc:
        with (
            tc.tile_pool(name="persist", bufs=1) as persist,
            tc.tile_pool(name="attn", bufs=8) as attn_pool,
            tc.tile_pool(name="rope", bufs=4) as rope_pool,
            tc.tile_pool(name="small", bufs=4) as small_pool,
            tc.tile_pool(name="yout", bufs=3) as yout_pool,
            tc.tile_pool(name="dram", bufs=1, space="DRAM") as dram_pool,
            tc.tile_pool(name="psA", bufs=2, space="PSUM") as psA,  # scores 2-bank
            tc.tile_pool(name="psB", bufs=2, space="PSUM") as psB,  # ctx accum
            tc.tile_pool(name="psC", bufs=2, space="PSUM") as psC,  # proj/y
        ):
            # ---------------- input DMA ----------------
            # few, large DMAs on the sync+gpsimd queues; ordered so the
            # tensors the pipeline needs first (wq, xt, cos/sin first half,
            # mask) land first instead of queueing behind the rest
            # ordered by first use: the critical sequence to reach steady
            # state is wq+xt+wk (projections), cos/sin chunk 0 + mask
            # (rope + first exp); wv/wo and the later cos/sin chunks ride
            # on the scalar queue / sync tail
            wq_sb = persist.tile([128, N_DC, DQ], bf16, tag="wq")
            nc.sync.dma_start(out=wq_sb[:], in_=wq[:])
            xt_sb = [
                persist.tile([128, S], bf16, tag=f"xt{dc}", name=f"xt{dc}")
                for dc in range(N_DC)
            ]
            cos_sb = persist.tile([128, N_SB, 512], f32, tag="cos")
            sin_sb = persist.tile([128, N_SB, 512], f32, tag="sin")
            mask_sb = persist.tile([128, 2, 128], bf16, tag="mask")
            wk_sb = persist.tile([128, N_DC, DQ], bf16, tag="wk")
            wv_sb = persist.tile([128, N_DC, DQ], bf16, tag="wv")
            wo_sb = persist.tile([128, 2, D], bf16, tag="wo")
            nc.gpsimd.dma_start(out=cos_sb[:, 0, :], in_=cosT[0])
            nc.gpsimd.dma_start(out=sin_sb[:, 0, :], in_=sinT[0])
            nc.gpsimd.dma_start(
                out=mask_sb[:], in_=masks.rearrange("p (h c) -> p h c", h=2)
            )
            for dc in (1, 3, 5, 7):
                nc.gpsimd.dma_start(
                    out=xt_sb[dc][:], in_=xT[128 * dc : 128 * (dc + 1), :]
                )
            for dc in (0, 2):
                nc.sync.dma_start(
                    out=xt_sb[dc][:], in_=xT[128 * dc : 128 * (dc + 1), :]
                )
            nc.sync.dma_start(out=wk_sb[:], in_=wk[:])
            for dc in (4, 6):
                nc.sync.dma_start(
                    out=xt_sb[dc][:], in_=xT[128 * dc : 128 * (dc + 1), :]
                )
            for c in range(1, N_SB):
                nc.sync.dma_start(out=cos_sb[:, c, :], in_=cosT[c])
                nc.sync.dma_start(out=sin_sb[:, c, :], in_=sinT[c])
            nc.scalar.dma_start(out=wv_sb[:], in_=wv[:])
            nc.scalar.dma_start(out=wo_sb[:], in_=wo[:])

            # PE warm-up: the HAM clock gate needs ~3.4us of sustained
            # activity to lift the PE to 2.4GHz, and one >3.4us idle window
            # re-throttles it. Run throwaway matmuls on a memset constant
            # tile before any input lands, and keep sprinkling short warm
            # bursts between prologue units so the input-arrival gaps never
            # open an idle window (the whole prologue would otherwise run at
            # 1.2GHz).
            cst_sb = persist.tile([128, DQ], bf16, tag="cst")
            nc.vector.memset(cst_sb[:], 0.5)
            warm0 = psA.tile([128, DQ], f32, tag="score", name="warm0")

            def warm(n, width=DQ):
                for _ in range(n):
                    nc.tensor.matmul(
                        warm0[:, 0:width],
                        cst_sb[:, 0:128],
                        cst_sb[:, 0:width],
                        start=True,
                        stop=True,
                    )

            warm(24)

            # persistent intermediates
            qT_sb = persist.tile([128, 2, S], bf16, tag="qT")  # [64h..., cc, s]
            kT_sb = persist.tile([128, 2, S], bf16, tag="kT")
            v_sb = persist.tile([128, N_KB, H_PER_CORE, HD + 1], bf16, tag="v")
            nc.vector.memset(v_sb[:, :, :, HD : HD + 1], 1.0)
            ctxT_sb = persist.tile([128, 2, S], bf16, tag="ctxT")  # unnormalized
            # denominators staged on one partition (engine writes must start at
            # partition 0/32/64/96); chunk qb*4+hh holds head hh, block qb
            stage_sb = persist.tile([1, H_PER_CORE * S], f32, tag="stage")
            recip_dram = dram_pool.tile([N_SB, H_PER_CORE, 512], bf16, tag="rdram")

            # ---------------- helpers ----------------
            def rope(src_ps, dst_sb, cc, sb):
                """dst = src*cos + rotate_half(src)*sin, fp32 in, bf16 out.

                The rotate-half partition shift is done by small SBUF->SBUF
                DMAs (a [32,512] DVE op costs as much as a [128,512] one, so
                quarter-sized DVE ops waste 3/4 of the lanes; DMA engines are
                otherwise idle).
                """
                t1 = rope_pool.tile([128, 512], bf16, tag="ropeA", name="t1")
                nc.vector.tensor_mul(t1[:], src_ps[:], cos_sb[:, sb, :])
                # sin table is pre-shifted on the host (sinx[p] =
                # sin_signed[partner(p)]) so this product is computed at the
                # SOURCE rows and only then moved to the partner rows by DMA
                t2p = rope_pool.tile([128, 512], bf16, tag="ropeQ", name="t2p")
                nc.vector.tensor_mul(t2p[:], src_ps[:], sin_sb[:, sb, :])
                ss = slice(512 * sb, 512 * (sb + 1))
                rot = rope_pool.tile([128, 512], bf16, tag="ropeB", name="rot")
                for quarter in range(4):
                    o = 32 * quarter
                    src_o = o + 32 if quarter % 2 == 0 else o - 32
                    nc.gpsimd.dma_start(
                        out=rot[o : o + 32, :], in_=t2p[src_o : src_o + 32, :]
                    )
                nc.vector.tensor_add(dst_sb[:, cc, ss], t1[:], rot[:])

            def proj_q(cc, sb):
                ss = slice(512 * sb, 512 * (sb + 1))
                q_ps = psC.tile([128, 512], f32, tag="proj", name="q_ps")
                for dc in range(N_DC):
                    nc.tensor.matmul(
                        q_ps[:],
                        wq_sb[:, dc, 128 * cc : 128 * (cc + 1)],
                        xt_sb[dc][:, ss],
                        start=(dc == 0),
                        stop=(dc == N_DC - 1),
                    )
                rope(q_ps, qT_sb, cc, sb)

            def proj_k(cc, sb):
                ss = slice(512 * sb, 512 * (sb + 1))
                k_ps = psC.tile([128, 512], f32, tag="proj", name="k_ps")
                for dc in range(N_DC):
                    nc.tensor.matmul(
                        k_ps[:],
                        wk_sb[:, dc, 128 * cc : 128 * (cc + 1)],
                        xt_sb[dc][:, ss],
                        start=(dc == 0),
                        stop=(dc == N_DC - 1),
                    )
                rope(k_ps, kT_sb, cc, sb)

            def proj_v(sc):
                v_ps = psC.tile([128, DQ], f32, tag="proj", name="v_ps")
                for dc in range(N_DC):
                    nc.tensor.matmul(
                        v_ps[:],
                        xt_sb[dc][:, 128 * sc : 128 * (sc + 1)],
                        wv_sb[:, dc, :],
                        start=(dc == 0),
                        stop=(dc == N_DC - 1),
                    )
                nc.vector.tensor_copy(
                    v_sb[:, sc, :, 0:HD],
                    v_ps[:].rearrange("p (h d) -> p h d", h=H_PER_CORE),
                )

            def out_proj(qb, oc):
                """Partial out-projection for query block qb, output chunk oc."""
                qs = slice(512 * qb, 512 * (qb + 1))
                y_ps = psC.tile([128, 512], f32, tag="proj", name="y_ps")
                for cc in range(2):
                    nc.tensor.matmul(
                        y_ps[:],
                        wo_sb[:, cc, 128 * oc : 128 * (oc + 1)],
                        ctxT_sb[:, cc, qs],
                        start=(cc == 0),
                        stop=(cc == 1),
                    )
                y_sb = yout_pool.tile([128, 512], bf16, tag="y", name="y_sb")
                nc.vector.tensor_copy(y_sb[:], y_ps[:])
                nc.sync.dma_start(
                    out=yT[128 * oc : 128 * (oc + 1), qs], in_=y_sb[:]
                )

            def attention(cc, qb, fillers=()):
                """Causal attention for head pair cc, query block qb.

                Per k-block: two score matmuls (head h in PE row-group h) into
                one [128,1024] PSUM tile, one exp over both heads, a
                triangular band mask on diagonal blocks, then (one k-block
                delayed) the two AV matmuls accumulating ctx+denominator via
                the ones column. Diagonal blocks narrow everything to the
                causally reachable q-range.

                `fillers` is a list of callables emitting independent PE
                work, spread between k-blocks to cover the exp-bound inner
                loop (the PE would otherwise idle ~0.5us per k-block).
                """
                qs0 = 512 * qb
                nkb = 4 * qb + 4
                fillers = list(fillers)
                emit_at = {}
                for i, f in enumerate(fillers):
                    kb_i = min(nkb - 1, 1 + (i * nkb) // max(1, len(fillers)))
                    emit_at.setdefault(kb_i, []).append(f)
                ctx_ps = [
                    psB.tile([HD + 1, 512], f32, tag="ctx", name=f"ctx{h}")
                    for h in range(2)
                ]
                pending = None  # (kb, lo, a_t) whose AV matmuls haven't run
                for kb in range(nkb):
                    diag = kb >= 4 * qb
                    lo = 128 * (kb - 4 * qb) if diag else 0
                    s_ps = psA.tile([128, 1024], f32, tag="score", name="s_ps")
                    for h in range(2):
                        hp = slice(64 * h, 64 * (h + 1))
                        nc.tensor.matmul(
                            s_ps[:, 512 * h + lo : 512 * (h + 1)],
                            kT_sb[hp, cc, 128 * kb : 128 * (kb + 1)],
                            qT_sb[hp, cc, qs0 + lo : qs0 + 512],
                            start=True,
                            stop=True,
                        )
                    a_t = attn_pool.tile(
                        [128, 2, 512], bf16, tag="attnT", name="a_t"
                    )
                    nc.scalar.activation(
                        a_t[:, :, lo:512],
                        s_ps[:].rearrange("p (h q) -> p h q", h=2)[:, :, lo:512],
                        mybir.ActivationFunctionType.Exp,
                        scale=float(1.0 / np.sqrt(HD)),
                    )
                    if diag:
                        nc.vector.tensor_mul(
                            a_t[:, :, lo : lo + 128],
                            a_t[:, :, lo : lo + 128],
                            mask_sb[:],
                        )
                    if pending is not None:
                        pkb, plo, p_t = pending
                        for h in range(2):
                            nc.tensor.matmul(
                                ctx_ps[h][:, plo:512],
                                v_sb[:, pkb, 2 * cc + h, :],
                                p_t[:, h, plo:512],
                                start=(pkb == 0),
                                stop=False,
                            )
                    for f in emit_at.get(kb, ()):
                        f()

                    pending = (kb, lo, a_t)
                pkb, plo, p_t = pending
                for h in range(2):
                    nc.tensor.matmul(
                        ctx_ps[h][:, plo:512],
                        v_sb[:, pkb, 2 * cc + h, :],
                        p_t[:, h, plo:512],
                        start=(pkb == 0),
                        stop=True,
                    )
                # stage denominators first (the normalization chain hangs
                # off them), then the bulk ctx copies
                r0 = qb * H_PER_CORE + 2 * cc
                nc.vector.tensor_copy(
                    stage_sb[0:1, 512 * r0 : 512 * (r0 + 1)],
                    ctx_ps[0][HD : HD + 1, :],
                )
                nc.scalar.copy(
                    stage_sb[0:1, 512 * (r0 + 1) : 512 * (r0 + 2)],
                    ctx_ps[1][HD : HD + 1, :],
                )
                for h in range(2):
                    nc.vector.tensor_copy(
                        ctxT_sb[64 * h : 64 * (h + 1), cc, qs0 : qs0 + 512],
                        ctx_ps[h][0:HD, :],
                    )

            def normalize(cc, qb, tail=False):
                """Reciprocal + broadcast + scale for head pair cc, block qb."""
                # repartition [1, 1024] -> [8, 128] so reciprocal is cheap
                # (reciprocal cost scales with free size only) and so the DVE
                # queue does not block on the scalar engine's stage copy (the
                # wait happens on the sync queue instead)
                base = (qb * H_PER_CORE + 2 * cc) * 512
                den_q = small_pool.tile([8, 128], f32, tag="den_q", name="den_q")
                # the final block's chain rides the scalar queue (empty once
                # the last exp retires) instead of the busy sync queue
                deng = nc.scalar if tail else nc.sync
                deng.dma_start(
                    out=den_q[:], in_=stage_sb[0:1, base : base + 1024]
                )
                rec_q = small_pool.tile([8, 128], bf16, tag="rec_q", name="rec_q")
                with nc.allow_low_precision(
                    reason="bf16 softmax denom matches bf16 attn weights"
                ):
                    nc.vector.reciprocal(rec_q[:], den_q[:])
                deng.dma_start(
                    out=recip_dram[qb, 2 * cc : 2 * cc + 2, :], in_=rec_q[:]
                )
                qs = slice(512 * qb, 512 * (qb + 1))
                bc_sb = small_pool.tile([128, 512], bf16, tag="bcast", name="bc_sb")
                for h in range(2):
                    row = recip_dram[qb, 2 * cc + h, :]
                    bcast = bass.AP(
                        tensor=row.tensor,
                        offset=row.offset,
                        ap=[[0, 64]] + list(row.ap)[-1:],
                    )
                    deng.dma_start(
                        out=bc_sb[64 * h : 64 * (h + 1), :], in_=bcast
                    )
                nc.vector.tensor_mul(
                    ctxT_sb[:, cc, qs], ctxT_sb[:, cc, qs], bc_sb[:]
                )

            # ---------------- main pipeline ----------------
            # prologue: just what attention(0, 0) needs; the rest of block
            # 0's projections ride as fillers inside attention(0, 0).
            # Warm bursts bridge the input-arrival gaps between units.
            proj_q(0, 0)
            warm(8, 128)
            proj_k(0, 0)
            warm(8, 128)
            for sc in range(4):
                proj_v(sc)
                warm(6, 128)

            for sb in range(N_SB):
                nxt = sb + 1
                # fillers for attention(0, sb): next block's cc=0 q/k
                # projections + v, then the previous block's out-projection
                fill0 = []
                if sb == 0:
                    fill0 += [lambda: proj_q(1, 0), lambda: proj_k(1, 0)]
                if nxt < N_SB:
                    fill0 += [
                        lambda s=nxt: proj_q(0, s),
                        lambda s=nxt: proj_k(0, s),
                        lambda s=nxt: proj_v(4 * s + 0),
                        lambda s=nxt: proj_v(4 * s + 1),
                    ]
                if sb >= 1:
                    fill0 += [
                        (lambda q=sb - 1, oc=oc: out_proj(q, oc))
                        for oc in range(0, 4)
                    ]
                attention(0, sb, fillers=fill0)
                normalize(0, sb)

                fill1 = []
                if nxt < N_SB:
                    fill1 += [
                        lambda s=nxt: proj_q(1, s),
                        lambda s=nxt: proj_k(1, s),
                        lambda s=nxt: proj_v(4 * s + 2),
                        lambda s=nxt: proj_v(4 * s + 3),
                    ]
                if sb >= 1:
                    fill1 += [
                        (lambda q=sb - 1, oc=oc: out_proj(q, oc))
                        for oc in range(4, N_DC)
                    ]
                attention(1, sb, fillers=fill1)
                normalize(1, sb, tail=(sb == N_SB - 1))

            for oc in range(N_DC):
                out_proj(N_SB - 1, oc)

    nc.compile()
    return nc


def _rope_tables():
    inv_freq = (
        1.0 / (THETA ** (np.arange(0, HD, 2, dtype=np.float32) / HD))
    ).astype(np.float32)
    pos = np.arange(S, dtype=np.float32)
    ang = pos[:, None] * inv_freq[None, :]  # [S, 32]
    cos_half = np.cos(ang).astype(np.float32).T  # [32, S]
    sin_half = np.sin(ang).astype(np.float32).T
    # per-head 64 rows: cos rows duplicated. The sin table is PRE-SHIFTED:
    # row p holds sin_signed[partner(p)] (partner = rotate-half swap), so the
    # kernel multiplies at the source rows and a plain partition-shift DMA
    # finishes rotate-half: sinx per head = (+sin | -sin).
    cos64 = np.concatenate([cos_half, cos_half], axis=0)
    sinx64 = np.concatenate([sin_half, -sin_half], axis=0)
    cosT = np.concatenate([cos64, cos64], axis=0)  # [128, S] two heads
    sinT = np.concatenate([sinx64, sinx64], axis=0)

    def chunked(t):  # [128, S] -> [N_SB, 128, 512] chunk-major
        return np.ascontiguousarray(
            t.reshape(128, S // 512, 512).transpose(1, 0, 2)
        )

    return chunked(cosT), chunked(sinT)


def _masks():
    k = np.arange(128)[:, None]
    c = np.arange(128)[None, :]
    tri = (k <= c).astype(ml_dtypes.bfloat16)  # [128, 128] band triangle
    return np.ascontiguousarray(np.concatenate([tri, tri], axis=1))


def kernel(x, W_q, W_k, W_v, W_o):
    global _CACHED
    from concourse.bass_utils import run_bass_kernel_spmd

    if _CACHED is None:
        _CACHED = _build_kernel()
    nc = _CACHED

    bf = ml_dtypes.bfloat16
    cosT, sinT = _rope_tables()
    masks = _masks()
    x = np.asarray(x)
    W_q, W_k, W_v, W_o = (np.asarray(w) for w in (W_q, W_k, W_v, W_o))
    xT = [np.ascontiguousarray(x[b].T).astype(bf) for b in range(B)]

    def chunk_major(w):  # [D_in, n] -> [128, D_in//128, n] (SBUF layout)
        w = np.asarray(w, dtype=bf)
        return np.ascontiguousarray(
            w.reshape(w.shape[0] // 128, 128, w.shape[1]).transpose(1, 0, 2)
        )

    in_maps = []
    for c in range(N_CORES):
        b, g = divmod(c, 4)
        cols = slice(DQ * g, DQ * (g + 1))
        in_maps.append(
            {
                "xT": xT[b],
                "wq": chunk_major(W_q[:, cols]),
                "wk": chunk_major(W_k[:, cols]),
                "wv": chunk_major(W_v[:, cols]),
                "wo": chunk_major(W_o[cols, :]),
                "cosT": cosT,
                "sinT": sinT,
                "masks": masks,
            }
        )

    res = run_bass_kernel_spmd(nc, in_maps, core_ids=list(range(N_CORES)))
    kernel.last_results = res

    y = np.empty((B, S, D), dtype=np.float32)
    for b in range(B):
        acc = res.results[4 * b]["yT"].astype(np.float32)
        for g in range(1, 4):
            acc += res.results[4 * b + g]["yT"].astype(np.float32)
        y[b] = acc.T
    return y
